# revision 4
# baseline (speedup 1.0000x reference)
"""Multi-head attention (RoPE + softmax) Trainium2 kernel, 8 NeuronCores, v2.

Sharding: B=2 batches x 16 heads -> each core owns one batch and 4 heads.
Wq/Wk/Wv split column-wise (by head), Wo row-wise; partial Wo-outputs
summed on the host.

v2 redesign vs baseline:
  - No [65,S] Q'/K' assembly: rope output stays as stacked 2-head
    [128,S] fp16 tiles; scores use 64-contraction matmuls with
    tile_position derived from the partition base (head even -> rows
    0-63 / tile (0,0); head odd -> rows 64-127 / tile (64,0)), so the
    two heads of a pair occupy disjoint PE row-groups and overlap.
  - Attention mask enters as per-partition (per-key) bias on the exp
    ACTIVATE instead of a K'-row, killing all SBUF->SBUF assembly DMAs.
  - One fluid schedule: V-projection and the next head-pair's QK
    projection + rope run as PE filler units interleaved into the
    attention kt loops, emitted before the iteration that consumes
    them (FIFO-safe).
  - Optional DVE exp offload (Schraudolph int16 bit-trick, bitcast to
    fp16) on a tunable subset of key-tiles to balance ACT vs PE.
  - Drain DMAs ride the SP/Pool queues, never the ACT queue.
"""

import os
import sys

for _p in ("/opt/trn_rl_repo", "/root/.axon_site/_ro/trn_rl_repo"):
    if os.path.isdir(_p) and _p not in sys.path:
        sys.path.insert(0, _p)

import contextlib

import numpy as np

import bass_rust
import concourse.bass as bass
import concourse.tile as tile
from concourse import mybir
from concourse.bass_utils import run_bass_kernel_spmd

B, S, H = 2, 2048, 1024
NH, HD = 16, 64
ROPE_BASE = 10000.0
N_CORES = 8
HPC = 4                # heads per core
DL = HPC * HD          # local head dims per core (256)
F32 = mybir.dt.float32
F32R = mybir.dt.float32r
FP16 = mybir.dt.float16
I16 = mybir.dt.int16
AF = mybir.ActivationFunctionType
ALU = mybir.AluOpType

KT = S // 128          # 16 key tiles of 128
QT = S // 512          # 4 query tiles of 512
HT = H // 128          # 8 hidden k-tiles

MASK_NEG = -60000.0

# Schraudolph fp16 exp: bits = round(alpha*s + beta); bitcast int16->fp16.
# The mantissa-linear chord 1+theta >= 2^theta overestimates, so center
# the +-3% sawtooth by subtracting 0.043*2^10 from the exponent bias.
SCH_ALPHA = 1024.0 / float(np.log(2.0))        # 1477.3197
SCH_BETA = 1024.0 * (15.0 - 0.0430)            # 15316.0

# key-tiles whose exp runs on DVE (bit-trick) instead of ACT, per
# (hp, qb) region. The (1,0)/(1,1)/(1,2) windows carry no PE filler and
# are ACT-bound, so shifting ~1/3 of their exps to the otherwise-idle
# DVE rebalances the pipeline.
# NOTE: measured on HW — the DVE int16 tensor_scalar path is ~15x
# slower than modeled (489us vs 288us end-to-end with 15 tiles
# offloaded); keep empty.
DVE_EXP = {}


def _split_multiwait(nc):
    """Split multi-wait instructions into single-wait NoOp chains."""
    n_new = 0
    for f in nc.m.functions:
        for b in f.blocks:
            il = b.instructions
            i = 0
            while i < len(il):
                ins = il[i]
                si = getattr(ins, "sync_info", None)
                if si is not None and si.on_wait is not None and len(si.on_wait) > 1:
                    waits = list(si.on_wait)
                    ups = list(si.on_update) if si.on_update else []
                    ins.sync_info = bass_rust.SyncInfo(on_wait=[waits[-1]], on_update=ups)
                    nops = []
                    for j, w in enumerate(waits[:-1]):
                        nop = bass_rust.InstNoOp(
                            name=f"{ins.name}-w{j}",
                            engine=ins.engine,
                            sync_info=bass_rust.SyncInfo(on_wait=[w], on_update=[]),
                            bass_nofuse=True,
                        )
                        nops.append(nop)
                    il[i:i] = nops
                    n_new += len(nops)
                    i += len(nops)
                i += 1
    return n_new


def _emit_body(nc, tc, d, phases=3):
    with contextlib.ExitStack() as ctx:
        const = ctx.enter_context(tc.tile_pool(name="const", bufs=1))

        # All big inputs are pre-packed on the host so every DMA reads
        # contiguous multi-KB per-partition lines. x is packed t-major
        # ([128, QT, HT, 512]) so one 1MB transfer delivers a full
        # 512-query column block across all hidden k-tiles.
        wqk_sb = const.tile([128, HT, 512], FP16, name="wqk_sb")
        nc.scalar.dma_start(out=wqk_sb[:], in_=d["wqkp"].rearrange("p (k m) -> p k m", k=HT)[:])
        x_sb = const.tile([128, QT, HT, 512], FP16, name="x_sb")
        xp_r = d["xp"].rearrange("p (t k m) -> p t k m", t=QT, k=HT)
        # first query-block arrives k-tile by k-tile so the first
        # projection chain can start on kt=0 immediately
        for kt in range(HT):
            nc.sync.dma_start(out=x_sb[:, 0, kt], in_=xp_r[:, 0, kt])
        perm_sb = const.tile([128, 128], FP16, name="perm_sb")
        nc.scalar.dma_start(out=perm_sb[:], in_=d["perm"][:])
        cos_sb = const.tile([128, S], F32, name="cos_sb")
        sin_sb = const.tile([128, S], F32, name="sin_sb")
        nc.scalar.dma_start(out=cos_sb[:, 0:512], in_=d["cosT"][:, 0:512])
        nc.scalar.dma_start(out=sin_sb[:, 0:512], in_=d["sinT"][:, 0:512])
        mcol_sb = const.tile([128, KT], F32, name="mcol_sb")
        nc.scalar.dma_start(out=mcol_sb[:], in_=d["mcol"][:])
        mcolB_sb = const.tile([128, KT], F32, name="mcolB_sb")
        nc.scalar.dma_start(out=mcolB_sb[:], in_=d["mcolB"][:])
        wv_sb = const.tile([128, HT, 260], FP16, name="wv_sb")
        nc.scalar.dma_start(out=wv_sb[:], in_=d["wvp"].rearrange("p (k m) -> p k m", k=HT)[:])
        for t in range(1, QT):
            tsl = bass.ts(t, 512)
            nc.sync.dma_start(out=x_sb[:, t], in_=xp_r[:, t])
            nc.scalar.dma_start(out=cos_sb[:, tsl], in_=d["cosT"][:, tsl])
            nc.scalar.dma_start(out=sin_sb[:, tsl], in_=d["sinT"][:, tsl])
        sel2_sb = const.tile([2, 128], FP16, name="sel2_sb")
        nc.scalar.dma_start(out=sel2_sb[:], in_=d["sel2"][:])
        wo_sb = const.tile([128, 2, H], FP16, name="wo_sb")
        nc.scalar.dma_start(out=wo_sb[:], in_=d["wop"].rearrange("p (k m) -> p k m", k=2)[:])

        # ---- persistent activations
        actp = ctx.enter_context(tc.tile_pool(name="actp", bufs=1))
        qr = [actp.tile([128, S], FP16, name=f"qr{hp}") for hp in range(2)]
        kr = [actp.tile([128, S], FP16, name=f"kr{hp}") for hp in range(2)]
        qkraw = [actp.tile([128, S], FP16, name=f"qkraw{m}") for m in range(4)]
        v_sb = [actp.tile([128, 260], FP16, name=f"v{t}") for t in range(KT)]
        o_sb = [actp.tile([128, S], F32, name=f"o{m}") for m in range(2)]
        onorm = [actp.tile([128, S], FP16, name=f"onorm{m}") for m in range(2)]
        sums = [actp.tile([2, S], F32, name=f"sums{m}") for m in range(2)]
        rec = [actp.tile([2, S], FP16, name=f"rec{m}") for m in range(2)]

        # ---- working pools
        # PSUM: pscr 2 banks + pst 4 banks + pop 2 banks = 8
        pscr = ctx.enter_context(tc.tile_pool(name="pscr", bufs=2, space="PSUM"))
        pst = ctx.enter_context(tc.tile_pool(name="pst", bufs=2, space="PSUM"))
        pop = ctx.enter_context(tc.tile_pool(name="pop", bufs=1, space="PSUM"))
        ep = ctx.enter_context(tc.tile_pool(name="ep", bufs=4))
        rtp = ctx.enter_context(tc.tile_pool(name="rtp", bufs=2))
        stgp = ctx.enter_context(tc.tile_pool(name="stgp", bufs=2))
        ysbp = ctx.enter_context(tc.tile_pool(name="ysbp", bufs=2))

        # ---------- unit emitters ----------
        def _rope(m, t):
            """q*cos + rot(q)*sin; the partition rotation runs as a PE
            permutation matmul (sign lives in sinT)."""
            tsl = bass.ts(t, 512)
            sh = pscr.tile([128, 512], F32, tag="s", name="sh")
            nc.tensor.matmul(sh[:], perm_sb[:], qkraw[m][:, tsl],
                             start=True, stop=True)
            t1 = rtp.tile([128, 512], F32, tag="t1", name="t1")
            nc.vector.tensor_mul(t1[:], qkraw[m][:, tsl], cos_sb[:, tsl])
            t2 = rtp.tile([128, 512], F32, tag="t2", name="t2")
            nc.vector.tensor_mul(t2[:], sh[:], sin_sb[:, tsl])
            dst = kr[m - 2] if m >= 2 else qr[m]
            nc.vector.tensor_add(dst[:, tsl], t1[:], t2[:])

        def proj_qk(m, t):
            """Project one [128,512] tile of q or k (m: 0=q01 1=q23 2=k01
            3=k23), then copy to qkraw and rope it into qr/kr."""
            tsl = bass.ts(t, 512)
            ps = pscr.tile([128, 512], F32, tag="s", name="ps")
            for kt in range(HT):
                nc.tensor.matmul(
                    ps[:], wqk_sb[:, kt, m * 128:(m + 1) * 128], x_sb[:, t, kt, :],
                    start=(kt == 0), stop=(kt == HT - 1),
                )
            nc.vector.tensor_copy(out=qkraw[m][:, tsl], in_=ps[:])
            _rope(m, t)

        def proj_qk_pair(hp, t):
            """Both q and k chains of a head pair, k-tile interleaved so
            the lead-in pipelines with per-kt x DMA arrivals."""
            tsl = bass.ts(t, 512)
            mq, mk = hp, 2 + hp
            ps_q = pscr.tile([128, 512], F32, tag="s", name="ps_q")
            ps_k = pscr.tile([128, 512], F32, tag="s", name="ps_k")
            for kt in range(HT):
                nc.tensor.matmul(
                    ps_q[:], wqk_sb[:, kt, mq * 128:(mq + 1) * 128],
                    x_sb[:, t, kt, :], start=(kt == 0), stop=(kt == HT - 1))
                nc.tensor.matmul(
                    ps_k[:], wqk_sb[:, kt, mk * 128:(mk + 1) * 128],
                    x_sb[:, t, kt, :], start=(kt == 0), stop=(kt == HT - 1))
            nc.vector.tensor_copy(out=qkraw[mq][:, tsl], in_=ps_q[:])
            nc.vector.tensor_copy(out=qkraw[mk][:, tsl], in_=ps_k[:])
            _rope(mk, t)
            _rope(mq, t)

        def proj_v(t):
            """Project one [128,260] v tile (keys t*128..) + ones columns."""
            pv = pscr.tile([128, 260], F32, tag="s", name="pv")
            t4, col = t // 4, (t % 4) * 128
            for kt in range(HT):
                nc.tensor.matmul(
                    pv[:], x_sb[:, t4, kt, col:col + 128], wv_sb[:, kt, :],
                    start=(kt == 0), stop=(kt == HT - 1),
                )
            nc.vector.tensor_copy(out=v_sb[t][:], in_=pv[:])
            for hh in range(HPC):
                nc.vector.memset(v_sb[t][:, hh * 65 + 64:hh * 65 + 65], 1.0)

        def attn(hp, qb, filler, tail=False):
            qsl = bass.ds(qb * 512, 512)
            oP = [pop.tile([65, 512], F32, tag=f"o{j}", name=f"oP{j}") for j in range(2)]
            dve_set = DVE_EXP.get((hp, qb), ())
            for kt in range(KT):
                st = pst.tile([128, 1024], F32, tag="st", name="st")
                for j in range(2):
                    pb = j * 64
                    nc.tensor.matmul(
                        st[:, j * 512:(j + 1) * 512],
                        kr[hp][pb:pb + 64, kt * 128:(kt + 1) * 128],
                        qr[hp][pb:pb + 64, qsl],
                        start=True, stop=True,
                    )
                e = ep.tile([128, 1024], FP16, tag="e", name="e")
                if kt in dve_set:
                    nc.vector.tensor_scalar(
                        out=e.bitcast(I16)[:], in0=st[:],
                        scalar1=SCH_ALPHA, scalar2=mcolB_sb[:, kt:kt + 1],
                        op0=ALU.mult, op1=ALU.add,
                    )
                else:
                    nc.scalar.activation(e[:], st[:], AF.Exp,
                                         bias=mcol_sb[:, kt:kt + 1])
                for j in range(2):
                    h = 2 * hp + j
                    nc.tensor.matmul(
                        oP[j][:], v_sb[kt][:, h * 65:h * 65 + 65],
                        e[:, j * 512:(j + 1) * 512],
                        start=(kt == 0), stop=(kt == KT - 1),
                    )
                next(filler, None)
            for j in range(2):
                h = 2 * hp + j
                stg = stgp.tile([65, 512], F32, tag="stg", name="stg")
                # hp0 windows are PE-bound with a busy DVE (rope/proj
                # fillers), so drain via the idle ACT; hp1 windows are
                # ACT-bound, so drain via the idle DVE.
                if hp == 0 or (tail and j == 1):
                    nc.scalar.copy(stg[:], oP[j][:])
                else:
                    nc.vector.tensor_copy(stg[:], oP[j][:])
                # sums first: they gate the reciprocal on the critical path
                nc.sync.dma_start(out=sums[hp][j:j + 1, qsl], in_=stg[64:65, :])
                nc.sync.dma_start(out=o_sb[hp][j * 64:j * 64 + 64, qsl],
                                  in_=stg[0:64, :])

        def rec_qb(m, qb):
            qsl = bass.ds(qb * 512, 512)
            # fp16 reciprocal output: 0.05% error on the softmax
            # denominator, lets the broadcast matmul run at fp16 rate
            with nc.allow_low_precision(reason="softmax denominator tolerates fp16"):
                nc.vector.reciprocal(rec[m][:, qsl], sums[m][:, qsl])

        def bc_qb(m, qb):
            qsl = bass.ds(qb * 512, 512)
            bc = pscr.tile([128, 512], F32, tag="s", name="bc")
            nc.tensor.matmul(bc[:], sel2_sb[:], rec[m][:, qsl],
                             start=True, stop=True)
            nc.vector.tensor_mul(onorm[m][:, qsl], o_sb[m][:, qsl], bc[:])

        def wo_block(mo, t, copy_eng):
            tsl = bass.ds(t * 512, 512)
            yp = pscr.tile([128, 512], F32, tag="s", name="yp")
            for k2 in range(2):
                nc.tensor.matmul(yp[:], wo_sb[:, k2, mo * 128:(mo + 1) * 128],
                                 onorm[k2][:, tsl],
                                 start=(k2 == 0), stop=(k2 == 1))
            ysb = ysbp.tile([128, 512], F32, tag="ysb", name="ysb")
            if copy_eng == "scalar":
                nc.scalar.copy(ysb[:], yp[:])
            else:
                nc.vector.tensor_copy(ysb[:], yp[:])
            nc.sync.dma_start(out=d["yT"][mo * 128:(mo + 1) * 128, tsl], in_=ysb[:])

        # ---------- schedule ----------
        def units(seq):
            for it in seq:
                if it is not None:
                    fn, *args = it
                    fn(*args)
                yield
            while True:
                yield

        def normwo_units(qb, tail=False):
            """norm + Wo filler units for one finished query block.

            As attention filler, keep all PSUM->SBUF copies on DVE (ACT
            has no slack mid-attention); in the tail, split them
            ACT/DVE since both are idle.
            """
            out = [(rec_qb, 0, qb), (rec_qb, 1, qb),
                   (bc_qb, 0, qb), (bc_qb, 1, qb)]
            out += [(wo_block, mo, qb,
                     ("scalar" if mo % 2 else "vector") if tail else "vector")
                    for mo in range(HT)]
            return out

        # lead-in: q01/k01 for queries 0-511, first 4 v tiles
        proj_qk_pair(0, 0)
        proj_v(0); proj_v(1); proj_v(2); proj_v(3)

        proj_units = (
            # hp0-qb0: remaining v tiles + k01 t1-3 (needed at kt 4/8/12)
            [(proj_v, 4), (proj_v, 5), (proj_qk, 2, 1), (proj_v, 6),
             (proj_v, 7), (proj_v, 8), (proj_qk, 2, 2), (proj_v, 9),
             (proj_v, 10), (proj_v, 11), (proj_qk, 2, 3), (proj_v, 12),
             (proj_v, 13), (proj_v, 14), (proj_v, 15), (proj_qk, 0, 1)]
            # hp0-qb1: all of q23/k23 (needed by hp1-qb0)
            + [(proj_qk, 3, 0), (proj_qk, 1, 0), (proj_qk, 3, 1),
               (proj_qk, 1, 1), (proj_qk, 3, 2), (proj_qk, 1, 2),
               (proj_qk, 3, 3), (proj_qk, 1, 3),
               None, None, None, None, None, None, None, None]
        )
        late_proj = [(proj_qk, 0, 2), (proj_qk, 0, 3)]
        if phases < 2:
            for it in proj_units:
                if it is not None:
                    fn, *args = it
                    fn(*args)
            return
        nw = (lambda qb: normwo_units(qb)) if phases >= 3 else (lambda qb: [None] * 12)
        fill = units(
            proj_units
            # hp1-qb0: late q01 projections (needed by hp0-qb2/qb3)
            + late_proj + [None] * 14
            # hp0-qb2: norm+Wo for query block 0
            + nw(0) + [None] * 4
            # hp1-qb1: nothing (qb1 completes at this window's end)
            + [None] * 16
            # hp0-qb3: norm+Wo for query block 1
            + nw(1) + [None] * 4
            # hp1-qb2: nothing (qb2 completes at this window's end)
            + [None] * 16
            # hp1-qb3: norm+Wo for query block 2
            + nw(2)
        )
        seq = ((0, 0), (0, 1), (1, 0), (0, 2), (1, 1), (0, 3), (1, 2), (1, 3))
        for i, (hp, qb) in enumerate(seq):
            attn(hp, qb, fill, tail=(i == len(seq) - 1))
        if phases < 3:
            return
        # tail: norm + Wo for the last query block
        for it in normwo_units(3, tail=True):
            fn, *args = it
            fn(*args)


def build(reps=1, split=True, phases=3):
    nc = bass.Bass("TRN2", target_bir_lowering=False, debug=False,
                   num_devices=N_CORES)
    d = {
        "xp": nc.dram_tensor("xp", [128, QT * HT * 512], FP16, kind="ExternalInput"),
        "wqkp": nc.dram_tensor("wqkp", [128, HT * 512], FP16, kind="ExternalInput"),
        "wvp": nc.dram_tensor("wvp", [128, HT * 260], FP16, kind="ExternalInput"),
        "wop": nc.dram_tensor("wop", [128, 2 * H], FP16, kind="ExternalInput"),
        "cosT": nc.dram_tensor("cosT", [128, S], F32, kind="ExternalInput"),
        "sinT": nc.dram_tensor("sinT", [128, S], F32, kind="ExternalInput"),
        "perm": nc.dram_tensor("perm", [128, 128], FP16, kind="ExternalInput"),
        "sel2": nc.dram_tensor("sel2", [2, 128], FP16, kind="ExternalInput"),
        "mcol": nc.dram_tensor("mcol", [128, KT], F32, kind="ExternalInput"),
        "mcolB": nc.dram_tensor("mcolB", [128, KT], F32, kind="ExternalInput"),
        "yT": nc.dram_tensor("yT", [H, S], F32, kind="ExternalOutput"),
    }
    with tile.TileContext(nc) as tc:
        if reps == 1:
            _emit_body(nc, tc, d, phases)
        else:
            with tc.For_i(0, reps, 1):
                _emit_body(nc, tc, d, phases)
    if split:
        _split_multiwait(nc)
    return nc


def host_inputs(x, attention_mask, Wq, Wk, Wv, Wo):
    """Build the 8 per-core input maps (numpy only)."""
    x = np.asarray(x, dtype=np.float32)
    attention_mask = np.asarray(attention_mask, dtype=np.float32)
    Wq = np.asarray(Wq, dtype=np.float32)
    Wk = np.asarray(Wk, dtype=np.float32)
    Wv = np.asarray(Wv, dtype=np.float32)
    Wo = np.asarray(Wo, dtype=np.float32)

    # x packed [128, (t4, kt, 512)]: row p holds, for each 512-query
    # block t4 and hidden k-tile kt, the x values of hidden dim kt*128+p.
    xp = [np.ascontiguousarray(
        x[b].T.reshape(HT, 128, QT, 512).transpose(1, 2, 0, 3).reshape(128, -1)
    ).astype(np.float16) for b in range(B)]

    p = np.arange(128)
    dd = p % HD
    inv = ROPE_BASE ** (-(dd % 32).astype(np.float32) / 32.0)
    s = np.arange(S, dtype=np.float32)
    ang = inv[:, None] * s[None, :]
    cosT = np.cos(ang).astype(np.float32)
    sinT = (np.where(dd < 32, -1.0, 1.0)[:, None] * np.sin(ang)).astype(np.float32)

    perm = np.zeros((128, 128), dtype=np.float16)
    for m in range(128):
        head, d_ = m // HD, m % HD
        perm[head * HD + (d_ + 32) % HD, m] = 1.0

    sel2 = np.zeros((2, 128), dtype=np.float16)
    sel2[0, 0:64] = 1.0
    sel2[1, 64:128] = 1.0

    # mask bias per key, tiled [128, KT]: bias = MASK_NEG * (1 - mask)
    mbias = [(MASK_NEG * (1.0 - attention_mask[b])).astype(np.float32)
             for b in range(B)]
    mcol = [np.ascontiguousarray(mb.reshape(KT, 128).T) for mb in mbias]
    mcolB = [np.ascontiguousarray(
        (SCH_BETA + SCH_ALPHA * mb).astype(np.float32).reshape(KT, 128).T)
        for mb in mbias]

    in_maps = []
    for c in range(N_CORES):
        b, hq = c // 4, (c % 4) * HPC
        d0 = hq * HD
        wqkT = np.ascontiguousarray(
            np.concatenate([Wq[d0:d0 + DL] * (HD ** -0.5), Wk[d0:d0 + DL]], axis=0).T
        ).astype(np.float16)
        wqkp = np.ascontiguousarray(
            wqkT.reshape(HT, 128, 512).transpose(1, 0, 2).reshape(128, -1))
        wvT = np.zeros((H, 260), dtype=np.float16)
        for hh in range(HPC):
            wvT[:, hh * 65:hh * 65 + 64] = Wv[d0 + hh * HD:d0 + (hh + 1) * HD].T
        wvp = np.ascontiguousarray(
            wvT.reshape(HT, 128, 260).transpose(1, 0, 2).reshape(128, -1))
        woT = np.ascontiguousarray(Wo[:, d0:d0 + DL].T).astype(np.float16)
        wop = np.ascontiguousarray(
            woT.reshape(2, 128, H).transpose(1, 0, 2).reshape(128, -1))
        in_maps.append({
            "xp": xp[b], "wqkp": wqkp, "wvp": wvp, "wop": wop,
            "cosT": cosT, "sinT": sinT, "perm": perm, "sel2": sel2,
            "mcol": mcol[b], "mcolB": mcolB[b],
        })
    return in_maps


def gather_output(results):
    y = np.zeros((B, S, H), dtype=np.float32)
    for c in range(N_CORES):
        y[c // 4] += results[c]["yT"].T
    return y


_nc_cache = {}


def kernel(x, attention_mask, Wq, Wk, Wv, Wo):
    if "nc" not in _nc_cache:
        _nc_cache["nc"] = build(reps=1)
    nc = _nc_cache["nc"]
    in_maps = host_inputs(x, attention_mask, Wq, Wk, Wv, Wo)
    res = run_bass_kernel_spmd(nc, in_maps, list(range(N_CORES)), trace=False)
    return gather_output(res.results)
